# revision 1
# baseline (speedup 1.0000x reference)
"""Trainium2 Bass kernel for nn_ContrastiveLoss_dot (MISA contrastive loss).

Math (reference):
  M[i,j,r,w] = <im[j,r,:], s[i,w,:]>            # (B,B,R,W) matchmap
  M_max      = max_r M                          # (B,B,W)
  sims[i,j]  = sum_{w<n_i} M_max / n_i          # (B,B)
  scores     = sims.T                           # scores[img j, cap i]
  loss = sum over off-diag of relu(m + scores - rowdiag)
       + sum over off-diag of relu(m + scores - coldiag)

Sharding: data-parallel over captions. Captions are length-balanced across
the 8 cores (snake-deal on sorted n_i) and only the valid words of each
caption are packed into the per-core word axis -- invalid words would be
masked out anyway, so we never compute them.  Per core on device:
  stage 1:  OUT[w, (j,r)] = sT.T @ imT  (packed words x 4608 regions),
            fused segmented max over r (36 regions/image) -> M_max[w, j]
  stage 2:  sims[slot, j] = mask.T @ M_max  (mask bakes packing + 1/n_i mean)
Host: gather the 8 (16,128) sims blocks, scatter rows back to caption order,
assemble scores, final (B,B)->scalar hinge loss (negligible work).

Hardware constraints that shaped the kernel (walrus rejects otherwise):
 * every instruction has ONE sync-wait slot (TPB EVENTS struct), and fused
   fp32/f32r matmuls put theirs on the S3_LW descriptor;
 * a DMA on a reused HW queue carries a queue-epoch wait, so the kernel
   issues exactly 8 DMAs total (params + 6 im super-tiles + out) -- each
   lands on a virgin queue and carries at most one data wait;
 * tiny "wait-absorber" matmuls take the im-DMA waits on the PE queue so
   real matmuls only ever need the psum-slot (DVE) wait.
"""

import sys

if "/opt/trn_rl_repo" not in sys.path:
    sys.path.insert(0, "/opt/trn_rl_repo")

import numpy as np

B, R, W, D = 128, 36, 60, 1024
N_CORES = 8
CAPS = B // N_CORES              # 16 captions per core
KT = D // 128                    # 8 contraction tiles
# images per super-tile (6 DMAs).  Ramped: a small first tile lets the PE
# start ~13us earlier (DMAs serialize; the first matmul waits on params +
# im tile 0).  Every psum chunk must keep N = imgs*36 in [256, 504] for
# full-rate fp32r, so chunks are 8..14 images.
# smallest tile first: the first matmul waits on params + im tile 0, so a
# small lead tile starts the PE sooner (all chunk shapes stay HW-proven;
# measured best vs both uniform [22...] and a 7-DMA [9, 20, ...] ramp)
IM_SPLIT = [18, 22, 22, 22, 22, 22]
IM_CHUNKS = {18: [9, 9], 22: [11, 11]}
MARGIN = 0.2

_CACHE = {}


def _build_nc(wt_tiles, mm_dtype_name="float32r"):
    """Bass program for one core; word axis = wt_tiles*128 packed words."""
    import concourse.tile as tile
    from concourse import bacc, mybir

    mmdt = getattr(mybir.dt, mm_dtype_name)
    f32 = mybir.dt.float32
    wpad = wt_tiles * 128
    # stage-2 caption slots padded 16 -> 128: fp32r matmuls must target all
    # four 32-col PE groups (col_grp == 0xf), i.e. M must be 128
    PF = KT * wpad + wt_tiles * 128    # params free size per partition

    # Bacc (not raw Bass): its compile() runs generate_event_semaphores,
    # which legalizes multi-wait instructions for walrus (1 wait/inst).
    nc = bacc.Bacc("TRN2", target_bir_lowering=False, debug=False,
                   num_devices=N_CORES)

    imT = nc.dram_tensor("imT", [D, B * R], mmdt, kind="ExternalInput").ap()
    # params[p, k*wpad + w]            = sT[k*128 + p, w]       (f32 bits)
    # params[p, KT*wpad + t*CAPS + m]  = mask[t*128 + p, m]
    params = nc.dram_tensor("params", [128, PF], mmdt,
                            kind="ExternalInput").ap()
    out = nc.dram_tensor("out", [CAPS, B], f32, kind="ExternalOutput").ap()

    with tile.TileContext(nc) as tc:
        with (
            tc.tile_pool(name="p_pool", bufs=1) as p_pool,
            tc.tile_pool(name="im_pool", bufs=1) as im_pool,
            tc.tile_pool(name="mx_pool", bufs=1) as mx_pool,
            tc.tile_pool(name="o_pool", bufs=1) as o_pool,
            tc.tile_pool(name="ps1", bufs=7, space="PSUM") as ps1_pool,
            tc.tile_pool(name="ps2", bufs=1, space="PSUM") as ps2_pool,
        ):
            # issue on the gpsimd DMA ring so it overlaps the im-tile DMAs
            # (all on nc.sync), instead of serializing ahead of them
            params_sb = p_pool.tile([128, PF], mmdt)
            nc.gpsimd.dma_start(params_sb[:], params[:])

            def sT_chunk(k, wt):       # [128, 128] lhsT for (k, word-tile)
                off = k * wpad + wt * 128
                return params_sb[:, off:off + 128]

            def mask_chunk(t):         # [128, 128] stage-2 lhsT (slots padded)
                off = KT * wpad + t * 128
                return params_sb[:, off:off + 128]

            mmax_sb = mx_pool.tile([128, wt_tiles, B], mmdt)  # words x images
            ps2 = ps2_pool.tile([128, B], f32)

            # stage 1: stream imT once (6 big DMAs); fused segmented max
            img0 = 0
            for gi, G in enumerate(IM_SPLIT):
                cols = G * R
                # unique tag per super-tile: ramped sizes would otherwise all
                # be padded to the largest tile (tag slots are sized to max)
                im_sb = im_pool.tile([128, KT, cols], mmdt, tag=f"im{gi}")
                nc.sync.dma_start(
                    im_sb[:],
                    imT.rearrange("(ko p) n -> p ko n", p=128)[
                        :, :, img0 * R:img0 * R + cols])
                # wait-absorber: takes the im-DMA wait on the PE queue (PE
                # instructions have a single sync-wait slot; real matmuls
                # below then only ever need the psum-slot wait).  M=128 and
                # even N per the fp32r ISA restrictions.
                nc.tensor.matmul(
                    ps2[:, 0:2], im_sb[:, 0, 0:128], im_sb[:, 0, 0:2],
                    start=True, stop=True)
                # split the super-tile into psum-bank chunks (proven shapes)
                c0 = 0
                for cn in IM_CHUNKS[G]:
                    for wt in range(wt_tiles):
                        ps = ps1_pool.tile([128, 14, R], f32, tag="ps")
                        for k in range(KT):
                            nc.tensor.matmul(
                                ps[:, :cn, :],
                                sT_chunk(k, wt),
                                im_sb[:, k, c0 * R:(c0 + cn) * R],
                                start=(k == 0),
                                stop=(k == KT - 1),
                            )
                        nc.vector.reduce_max(
                            mmax_sb[:, wt, img0 + c0:img0 + c0 + cn],
                            ps[:, :cn, :],
                            axis=mybir.AxisListType.X,
                        )
                    c0 += cn
                img0 += G

            # stage 2: sims[slot, j] = sum_w mask[w, slot] * M_max[w, j]
            for wt in range(wt_tiles):
                nc.tensor.matmul(
                    ps2[:],
                    mask_chunk(wt),
                    mmax_sb[:, wt, :],
                    start=(wt == 0),
                    stop=(wt == wt_tiles - 1),
                )
            out_sb = o_pool.tile([CAPS, B], f32)
            nc.vector.tensor_copy(out=out_sb[:], in_=ps2[0:CAPS, :])
            nc.sync.dma_start(out[:], out_sb[:])

    nc.compile()
    return nc


def get_nc(wt_tiles, mm_dtype_name="float32r"):
    key = ("nc", wt_tiles, mm_dtype_name)
    if key not in _CACHE:
        _CACHE[key] = _build_nc(wt_tiles, mm_dtype_name)
    return _CACHE[key]


def assign_captions(s_l):
    """Length-balanced snake assignment: caps_of_core[c] = 16 caption ids."""
    n = np.asarray(s_l).astype(np.int64)
    order = np.argsort(-n, kind="stable")  # longest first
    caps_of_core = [[] for _ in range(N_CORES)]
    for r in range(CAPS):
        chunk = order[r * N_CORES:(r + 1) * N_CORES]
        cores = range(N_CORES) if r % 2 == 0 else range(N_CORES - 1, -1, -1)
        for c, cap in zip(cores, chunk):
            caps_of_core[c].append(int(cap))
    return caps_of_core


def make_core_inputs(im, s, s_l):
    """Host-side shard prep. Returns (in_maps, caps_of_core, wt_tiles)."""
    im = np.ascontiguousarray(im, dtype=np.float32)
    s = np.ascontiguousarray(s, dtype=np.float32)
    n = np.asarray(s_l).astype(np.int64)

    caps_of_core = assign_captions(s_l)
    packed = [int(sum(n[i] for i in caps)) for caps in caps_of_core]
    wt_tiles = max(1, -(-max(packed) // 128))  # ceil to 128
    wpad = wt_tiles * 128

    imT = np.ascontiguousarray(im.reshape(B * R, D).T)  # (1024, 4608)
    in_maps = []
    for c in range(N_CORES):
        sT = np.zeros((D, wpad), dtype=np.float32)
        mask = np.zeros((wpad, 128), dtype=np.float32)  # slots padded to 128
        off = 0
        for slot, cap in enumerate(caps_of_core[c]):
            ni = int(n[cap])
            sT[:, off:off + ni] = s[cap, :ni, :].T
            mask[off:off + ni, slot] = 1.0 / ni
            off += ni
        # pack sT + mask into the single params tensor, laid out exactly as
        # the SBUF tile reads it (partition-major)
        PF = KT * wpad + wt_tiles * 128
        pa = np.empty((128, PF), dtype=np.float32)
        # sT part: pa[p, k*wpad + w] = sT[k*128 + p, w]
        pa[:, :KT * wpad] = (
            sT.reshape(KT, 128, wpad).transpose(1, 0, 2).reshape(128, -1))
        # mask part: pa[p, KT*wpad + t*128 + m] = mask[t*128 + p, m]
        pa[:, KT * wpad:] = (
            mask.reshape(wt_tiles, 128, 128)
            .transpose(1, 0, 2).reshape(128, -1))
        in_maps.append({"imT": imT, "params": pa})
    return in_maps, caps_of_core, wt_tiles


def loss_from_sims(sims_all):
    """sims_all: (B, B) with sims[i cap, j img]; returns scalar loss."""
    scores = sims_all.T.astype(np.float64)  # scores[img j, cap i]
    diag = np.diag(scores).copy()
    cost_s = np.maximum(MARGIN + scores - diag[:, None], 0.0)
    cost_im = np.maximum(MARGIN + scores - diag[None, :], 0.0)
    np.fill_diagonal(cost_s, 0.0)
    np.fill_diagonal(cost_im, 0.0)
    return np.array(cost_s.sum() + cost_im.sum(), dtype=np.float32)


def kernel(im, s, s_l, x, _trace=False, _mm_dtype="float32r"):
    from concourse.bass_utils import run_bass_kernel_spmd

    in_maps, caps_of_core, wt_tiles = make_core_inputs(im, s, s_l)
    nc = get_nc(wt_tiles, _mm_dtype)
    res = run_bass_kernel_spmd(nc, in_maps, list(range(N_CORES)), trace=_trace)
    sims_all = np.zeros((B, B), dtype=np.float32)
    for c in range(N_CORES):
        block = res.results[c]["out"]  # (16, 128) rows in slot order
        for slot, cap in enumerate(caps_of_core[c]):
            sims_all[cap] = block[slot]
    loss = loss_from_sims(sims_all)
    if _trace:
        return loss, res
    return loss



# revision 2
# speedup vs baseline: 1.8057x; 1.8057x over previous
"""Trainium2 Bass kernel for nn_ContrastiveLoss_dot (MISA contrastive loss).

Math (reference):
  M[i,j,r,w] = <im[j,r,:], s[i,w,:]>            # (B,B,R,W) matchmap
  M_max      = max_r M                          # (B,B,W)
  sims[i,j]  = sum_{w<n_i} M_max / n_i          # (B,B)
  scores     = sims.T                           # scores[img j, cap i]
  loss = sum over off-diag of relu(m + scores - rowdiag)
       + sum over off-diag of relu(m + scores - coldiag)

Sharding: data-parallel over captions. Captions are length-balanced across
the 8 cores (snake-deal on sorted n_i) and only the valid words of each
caption are packed into the per-core word axis -- invalid words would be
masked out anyway, so we never compute them.  Per core on device:
  stage 1:  OUT[w, (j,r)] = sT.T @ imT  (packed words x 4608 regions) as
            fp8-e4m3 DoubleRow matmuls (2 k-tiles per instruction), fused
            segmented max over r (36 regions/image) -> M_max[w, j]
  stage 2:  sims[slot, j] = mask.T @ M_max  (fp32 mask bakes packing + 1/n_i)
Host: quantize im/s to e4m3 (loss rel-err ~4e-3, tolerance 2e-2), gather the
8 (16,128) sims blocks, scatter rows back to caption order, assemble scores,
final (B,B)->scalar hinge loss (negligible work).

Hardware constraints that shaped the kernel (walrus rejects otherwise):
 * every instruction has ONE sync-wait slot (TPB EVENTS struct); tiny
   "wait-absorber" matmuls take the im-DMA waits on the PE queue so real
   matmuls only ever need the psum-slot (DVE) wait;
 * a DMA on a reused HW queue carries a queue-epoch wait, so the kernel
   issues few DMAs (2 params + 6 im super-tiles + out), each on a virgin
   queue;
 * DoubleRow matmuls need lhsT/rhs APs shaped [K, 2, M]/[K, 2, N] (two
   k-tiles strided in dim 1) and pay ~72% extra LDWEIGHTS, so chunks use
   the full 504-column PSUM bank to amortize the weight (re)loads.
"""

import sys

if "/opt/trn_rl_repo" not in sys.path:
    sys.path.insert(0, "/opt/trn_rl_repo")

import numpy as np

B, R, W, D = 128, 36, 60, 1024
N_CORES = 8
CAPS = B // N_CORES              # 16 captions per core
KT = D // 128                    # 8 contraction tiles
KP = KT // 2                     # 4 DoubleRow k-pair passes
# images per super-tile (6 DMAs), ramped: a small first tile lets the PE
# start sooner.  Each psum chunk is <= 14 images (N = imgs*36 <= 504, one
# PSUM bank).  10 chunks total (minimum at 14-img banks) to amortize the
# DoubleRow LDWEIGHTS overhead.
IM_SPLIT = [10, 26, 26, 26, 26, 14]
IM_CHUNKS = {10: [10], 26: [13, 13], 14: [14]}
MARGIN = 0.2

_CACHE = {}


def _build_nc(wt_tiles):
    """Bass program for one core; word axis = wt_tiles*128 packed words."""
    import concourse.tile as tile
    from concourse import bacc, mybir

    fp8 = mybir.dt.float8e4
    f32r = mybir.dt.float32r
    f32 = mybir.dt.float32
    DR = mybir.MatmulPerfMode.DoubleRow
    wpad = wt_tiles * 128

    # Bacc (not raw Bass): its compile() runs generate_event_semaphores,
    # which legalizes multi-wait instructions for walrus (1 wait/inst).
    nc = bacc.Bacc("TRN2", target_bir_lowering=False, debug=False,
                   num_devices=N_CORES)

    imT = nc.dram_tensor("imT", [D, B * R], fp8, kind="ExternalInput").ap()
    # sT8[p, k*wpad + w] = sT[k*128 + p, w]   (e4m3)
    sT8 = nc.dram_tensor("sT8", [128, KT * wpad], fp8,
                         kind="ExternalInput").ap()
    # maskp[p, t*128 + m] = mask[t*128 + p, m]  (f32; bakes packing + 1/n)
    maskp = nc.dram_tensor("maskp", [128, wt_tiles * 128], f32r,
                           kind="ExternalInput").ap()
    out = nc.dram_tensor("out", [CAPS, B], f32, kind="ExternalOutput").ap()

    with tile.TileContext(nc) as tc:
        with (
            tc.tile_pool(name="p_pool", bufs=1) as p_pool,
            tc.tile_pool(name="m_pool", bufs=1) as m_pool,
            tc.tile_pool(name="im_pool", bufs=1) as im_pool,
            tc.tile_pool(name="mx_pool", bufs=1) as mx_pool,
            tc.tile_pool(name="o_pool", bufs=1) as o_pool,
            tc.tile_pool(name="ps1", bufs=7, space="PSUM") as ps1_pool,
            tc.tile_pool(name="ps2", bufs=1, space="PSUM") as ps2_pool,
        ):
            # issue on the gpsimd DMA ring so they overlap the im-tile DMAs
            # (all on nc.sync), instead of serializing ahead of them
            sT_sb = p_pool.tile([128, KT, wpad], fp8)
            nc.gpsimd.dma_start(
                sT_sb[:], sT8.rearrange("p (k w) -> p k w", k=KT))
            mask_sb = m_pool.tile([128, wt_tiles, 128], f32r)
            nc.gpsimd.dma_start(
                mask_sb[:], maskp.rearrange("p (t m) -> p t m", t=wt_tiles))

            mmax_sb = mx_pool.tile([128, wt_tiles, B], f32r)  # words x images
            ps2 = ps2_pool.tile([128, B], f32)

            # stage 1: stream imT once (6 DMAs); fused segmented max
            img0 = 0
            for gi, G in enumerate(IM_SPLIT):
                cols = G * R
                # unique tag per super-tile: ramped sizes would otherwise all
                # be padded to the largest tile (tag slots are sized to max)
                im_sb = im_pool.tile([128, KT, cols], fp8, tag=f"im{gi}")
                nc.sync.dma_start(
                    im_sb[:],
                    imT.rearrange("(ko p) n -> p ko n", p=128)[
                        :, :, img0 * R:img0 * R + cols])
                # wait-absorber: takes the im-DMA wait on the PE queue (PE
                # instructions have a single sync-wait slot; real matmuls
                # below then only ever need the psum-slot wait).
                nc.tensor.matmul(
                    ps2[:, 0:2], im_sb[:, 0, 0:128], im_sb[:, 0, 0:2],
                    start=True, stop=True)
                # split the super-tile into one-bank psum chunks
                c0 = 0
                for cn in IM_CHUNKS[G]:
                    for wt in range(wt_tiles):
                        ps = ps1_pool.tile([128, 14, R], f32, tag="ps")
                        for kp in range(KP):
                            nc.tensor.matmul(
                                ps[:, :cn, :],
                                sT_sb[:, 2 * kp:2 * kp + 2,
                                      wt * 128:wt * 128 + 128],
                                im_sb[:, 2 * kp:2 * kp + 2,
                                      c0 * R:(c0 + cn) * R],
                                start=(kp == 0),
                                stop=(kp == KP - 1),
                                perf_mode=DR,
                            )
                        nc.vector.reduce_max(
                            mmax_sb[:, wt, img0 + c0:img0 + c0 + cn],
                            ps[:, :cn, :],
                            axis=mybir.AxisListType.X,
                        )
                    c0 += cn
                img0 += G

            # stage 2: sims[slot, j] = sum_w mask[w, slot] * M_max[w, j]
            for wt in range(wt_tiles):
                nc.tensor.matmul(
                    ps2[:],
                    mask_sb[:, wt, :],
                    mmax_sb[:, wt, :],
                    start=(wt == 0),
                    stop=(wt == wt_tiles - 1),
                )
            out_sb = o_pool.tile([CAPS, B], f32)
            nc.vector.tensor_copy(out=out_sb[:], in_=ps2[0:CAPS, :])
            nc.sync.dma_start(out[:], out_sb[:])

    nc.compile()
    return nc


def get_nc(wt_tiles):
    key = ("nc", wt_tiles)
    if key not in _CACHE:
        _CACHE[key] = _build_nc(wt_tiles)
    return _CACHE[key]


def assign_captions(s_l):
    """Length-balanced snake assignment: caps_of_core[c] = 16 caption ids."""
    n = np.asarray(s_l).astype(np.int64)
    order = np.argsort(-n, kind="stable")  # longest first
    caps_of_core = [[] for _ in range(N_CORES)]
    for r in range(CAPS):
        chunk = order[r * N_CORES:(r + 1) * N_CORES]
        cores = range(N_CORES) if r % 2 == 0 else range(N_CORES - 1, -1, -1)
        for c, cap in zip(cores, chunk):
            caps_of_core[c].append(int(cap))
    return caps_of_core


def make_core_inputs(im, s, s_l):
    """Host-side shard prep. Returns (in_maps, caps_of_core, wt_tiles)."""
    import ml_dtypes

    e4m3 = ml_dtypes.float8_e4m3fn
    im = np.ascontiguousarray(im, dtype=np.float32)
    s = np.ascontiguousarray(s, dtype=np.float32)
    n = np.asarray(s_l).astype(np.int64)

    caps_of_core = assign_captions(s_l)
    packed = [int(sum(n[i] for i in caps)) for caps in caps_of_core]
    wt_tiles = max(1, -(-max(packed) // 128))  # ceil to 128
    wpad = wt_tiles * 128

    imT8 = np.ascontiguousarray(
        im.reshape(B * R, D).T.astype(e4m3))  # (1024, 4608) e4m3
    in_maps = []
    for c in range(N_CORES):
        sT = np.zeros((D, wpad), dtype=np.float32)
        mask = np.zeros((wpad, 128), dtype=np.float32)  # slots padded to 128
        off = 0
        for slot, cap in enumerate(caps_of_core[c]):
            ni = int(n[cap])
            sT[:, off:off + ni] = s[cap, :ni, :].T
            mask[off:off + ni, slot] = 1.0 / ni
            off += ni
        # sT8[p, k*wpad + w] = sT[k*128 + p, w]  (e4m3)
        sT8 = np.ascontiguousarray(
            sT.reshape(KT, 128, wpad).transpose(1, 0, 2).reshape(128, -1)
            .astype(e4m3))
        # maskp[p, t*128 + m] = mask[t*128 + p, m]
        maskp = np.ascontiguousarray(
            mask.reshape(wt_tiles, 128, 128)
            .transpose(1, 0, 2).reshape(128, -1))
        in_maps.append({"imT": imT8, "sT8": sT8, "maskp": maskp})
    return in_maps, caps_of_core, wt_tiles


def loss_from_sims(sims_all):
    """sims_all: (B, B) with sims[i cap, j img]; returns scalar loss."""
    scores = sims_all.T.astype(np.float64)  # scores[img j, cap i]
    diag = np.diag(scores).copy()
    cost_s = np.maximum(MARGIN + scores - diag[:, None], 0.0)
    cost_im = np.maximum(MARGIN + scores - diag[None, :], 0.0)
    np.fill_diagonal(cost_s, 0.0)
    np.fill_diagonal(cost_im, 0.0)
    return np.array(cost_s.sum() + cost_im.sum(), dtype=np.float32)


def kernel(im, s, s_l, x, _trace=False, _mm_dtype=None):
    from concourse.bass_utils import run_bass_kernel_spmd

    in_maps, caps_of_core, wt_tiles = make_core_inputs(im, s, s_l)
    nc = get_nc(wt_tiles)
    res = run_bass_kernel_spmd(nc, in_maps, list(range(N_CORES)), trace=_trace)
    sims_all = np.zeros((B, B), dtype=np.float32)
    for c in range(N_CORES):
        block = res.results[c]["out"]  # (16, 128) rows in slot order
        for slot, cap in enumerate(caps_of_core[c]):
            sims_all[cap] = block[slot]
    loss = loss_from_sims(sims_all)
    if _trace:
        return loss, res
    return loss


# revision 3
# speedup vs baseline: 1.9475x; 1.0786x over previous
"""Trainium2 Bass kernel for nn_ContrastiveLoss_dot (MISA contrastive loss).

Math (reference):
  M[i,j,r,w] = <im[j,r,:], s[i,w,:]>            # (B,B,R,W) matchmap
  M_max      = max_r M                          # (B,B,W)
  sims[i,j]  = sum_{w<n_i} M_max / n_i          # (B,B)
  scores     = sims.T                           # scores[img j, cap i]
  loss = sum over off-diag of relu(m + scores - rowdiag)
       + sum over off-diag of relu(m + scores - coldiag)

Sharding: data-parallel over captions. Captions are length-balanced across
the 8 cores (snake-deal on sorted n_i) and only the valid words of each
caption are packed into the per-core word axis -- invalid words would be
masked out anyway, so we never compute them.  Per core on device:
  stage 1:  OUT[w, (j,r)] = sT.T @ imT  (packed words x 4608 regions) as
            fp8-e4m3 DoubleRow matmuls (2 k-tiles per instruction), then a
            segmented max over r (36 regions/image) -> M_max[w, j] in bf16.
            The max is split across engines: half the chunks reduce
            directly on DVE; the other half are evicted PSUM->SBUF-bf16 by
            the scalar engine and reduced on DVE in a fast 16-bit mode.
  stage 2:  simsraw[slot, j] = mask01.T @ M_max  (bf16 0/1 mask; exact)
Host: quantize im/s to e4m3 (loss rel-err ~4e-3, tolerance 2e-2), gather the
8 (16,128) simsraw blocks, divide by n_i exactly, scatter rows back to
caption order, assemble scores, final (B,B)->scalar hinge loss.

Hardware constraints that shaped the kernel (walrus rejects otherwise):
 * every instruction has ONE sync-wait slot (TPB EVENTS struct); tiny
   "wait-absorber" matmuls take the im-DMA waits on the PE queue so real
   matmuls only ever need the psum-slot wait;
 * im is laid out supertile-major on host so each im DMA is one contiguous
   multi-KB run per partition (small strided runs drop DMA to ~200 GB/s);
 * params ride the scalar-engine HW-DGE ring: off the sync ring (which
   carries the im tiles) and off gpsimd (software DGE is slow);
 * DoubleRow matmuls need lhsT/rhs APs shaped [K, 2, M]/[K, 2, N] (two
   k-tiles strided in dim 1); chunks fill a 504-col PSUM bank to amortize
   the (mostly hidden) 256-col LDWEIGHTS.
"""

import sys

if "/opt/trn_rl_repo" not in sys.path:
    sys.path.insert(0, "/opt/trn_rl_repo")

import numpy as np

B, R, W, D = 128, 36, 60, 1024
N_CORES = 8
CAPS = B // N_CORES              # 16 captions per core
KT = D // 128                    # 8 contraction tiles
KP = KT // 2                     # 4 DoubleRow k-pair passes
# images per super-tile (6 DMAs), ramped: a small first tile lets the PE
# start sooner.  Each psum chunk is <= 14 images (N = imgs*36 <= 504, one
# PSUM bank); 10 chunks total.
IM_SPLIT = [8, 26, 26, 26, 28, 14]
IM_CHUNKS = {8: [8], 26: [13, 13], 28: [14, 14], 14: [14]}
MARGIN = 0.2

_CACHE = {}


def _build_nc(wt_tiles):
    """Bass program for one core; word axis = wt_tiles*128 packed words."""
    import concourse.tile as tile
    from concourse import bacc, mybir

    fp8 = mybir.dt.float8e4
    bf16 = mybir.dt.bfloat16
    f32 = mybir.dt.float32
    DR = mybir.MatmulPerfMode.DoubleRow
    wpad = wt_tiles * 128
    IM_LEN = KT * B * R  # flattened supertile-major im length per partition

    # Bacc (not raw Bass): its compile() runs generate_event_semaphores,
    # which legalizes multi-wait instructions for walrus (1 wait/inst).
    nc = bacc.Bacc("TRN2", target_bir_lowering=False, debug=False,
                   num_devices=N_CORES)

    # im_lin[p, off_g + k*cols_g + nn] = e4m3(im[j, r, k*128+p]),
    # nn = (j - j0_g)*R + r  -- supertile-major so each DMA is contiguous
    im_lin = nc.dram_tensor("im_lin", [128, IM_LEN], fp8,
                            kind="ExternalInput").ap()
    # sT8[p, k*wpad + w] = e4m3(sT[k*128 + p, w])
    sT8 = nc.dram_tensor("sT8", [128, KT * wpad], fp8,
                         kind="ExternalInput").ap()
    # maskb[p, t*128 + m] = bf16 0/1 word->slot membership
    maskb = nc.dram_tensor("maskb", [128, wt_tiles * 128], bf16,
                           kind="ExternalInput").ap()
    out = nc.dram_tensor("out", [CAPS, B], f32, kind="ExternalOutput").ap()

    with tile.TileContext(nc) as tc:
        with (
            tc.tile_pool(name="p_pool", bufs=1) as p_pool,
            tc.tile_pool(name="m_pool", bufs=1) as m_pool,
            tc.tile_pool(name="im_pool", bufs=1) as im_pool,
            tc.tile_pool(name="mx_pool", bufs=1) as mx_pool,
            tc.tile_pool(name="st_pool", bufs=3) as st_pool,
            tc.tile_pool(name="o_pool", bufs=1) as o_pool,
            tc.tile_pool(name="ps1", bufs=7, space="PSUM") as ps1_pool,
            tc.tile_pool(name="ps2", bufs=1, space="PSUM") as ps2_pool,
        ):
            # params on the scalar HW-DGE ring: overlaps the im DMAs (sync
            # ring) without serializing ahead of them
            sT_sb = p_pool.tile([128, KT, wpad], fp8)
            nc.scalar.dma_start(
                sT_sb[:], sT8.rearrange("p (k w) -> p k w", k=KT))
            mask_sb = m_pool.tile([128, wt_tiles, 128], bf16)
            nc.scalar.dma_start(
                mask_sb[:], maskb.rearrange("p (t m) -> p t m", t=wt_tiles))

            mmax_sb = mx_pool.tile([128, wt_tiles, B], bf16)  # words x images
            ps2 = ps2_pool.tile([128, B], f32)

            # stage 1: stream im once (6 DMAs); fused segmented max
            img0 = 0
            off = 0
            for gi, G in enumerate(IM_SPLIT):
                cols = G * R
                # unique tag per super-tile: ramped sizes would otherwise all
                # be padded to the largest tile (tag slots are sized to max)
                im_sb = im_pool.tile([128, KT, cols], fp8, tag=f"im{gi}")
                nc.sync.dma_start(
                    im_sb[:],
                    im_lin[:, off:off + KT * cols].rearrange(
                        "p (k n) -> p k n", k=KT))
                # wait-absorber: takes the im-DMA wait on the PE queue (PE
                # instructions have a single sync-wait slot; real matmuls
                # below then only ever need the psum-slot wait).
                nc.tensor.matmul(
                    ps2[:, 0:2], im_sb[:, 0, 0:128], im_sb[:, 0, 0:2],
                    start=True, stop=True)
                # split the super-tile into one-bank psum chunks
                c0 = 0
                for cn in IM_CHUNKS[G]:
                    for wt in range(wt_tiles):
                        ps = ps1_pool.tile([128, 14, R], f32, tag="ps")
                        for kp in range(KP):
                            nc.tensor.matmul(
                                ps[:, :cn, :],
                                sT_sb[:, 2 * kp:2 * kp + 2,
                                      wt * 128:wt * 128 + 128],
                                im_sb[:, 2 * kp:2 * kp + 2,
                                      c0 * R:(c0 + cn) * R],
                                start=(kp == 0),
                                stop=(kp == KP - 1),
                                perf_mode=DR,
                            )
                        dst = mmax_sb[:, wt, img0 + c0:img0 + c0 + cn]
                        if wt < 2:
                            # direct fp32 reduce on DVE
                            nc.vector.reduce_max(
                                dst, ps[:, :cn, :], axis=mybir.AxisListType.X)
                        else:
                            # scalar engine evicts PSUM -> SBUF bf16, DVE
                            # reduces in a 16-bit fast mode
                            stg = st_pool.tile([128, 14, R], bf16, tag="stg")
                            nc.scalar.copy(stg[:, :cn, :], ps[:, :cn, :])
                            nc.vector.reduce_max(
                                dst, stg[:, :cn, :], axis=mybir.AxisListType.X)
                    c0 += cn
                img0 += G
                off += KT * cols

            # stage 2: simsraw[slot, j] = sum_w mask01[w, slot] * M_max[w, j]
            for wt in range(wt_tiles):
                nc.tensor.matmul(
                    ps2[:],
                    mask_sb[:, wt, :],
                    mmax_sb[:, wt, :],
                    start=(wt == 0),
                    stop=(wt == wt_tiles - 1),
                )
            out_sb = o_pool.tile([CAPS, B], f32)
            nc.vector.tensor_copy(out=out_sb[:], in_=ps2[0:CAPS, :])
            nc.sync.dma_start(out[:], out_sb[:])

    nc.compile()
    return nc


def get_nc(wt_tiles):
    key = ("nc", wt_tiles)
    if key not in _CACHE:
        _CACHE[key] = _build_nc(wt_tiles)
    return _CACHE[key]


def assign_captions(s_l):
    """Length-balanced snake assignment: caps_of_core[c] = 16 caption ids."""
    n = np.asarray(s_l).astype(np.int64)
    order = np.argsort(-n, kind="stable")  # longest first
    caps_of_core = [[] for _ in range(N_CORES)]
    for r in range(CAPS):
        chunk = order[r * N_CORES:(r + 1) * N_CORES]
        cores = range(N_CORES) if r % 2 == 0 else range(N_CORES - 1, -1, -1)
        for c, cap in zip(cores, chunk):
            caps_of_core[c].append(int(cap))
    return caps_of_core


def make_core_inputs(im, s, s_l):
    """Host-side shard prep. Returns (in_maps, caps_of_core, wt_tiles)."""
    import ml_dtypes

    e4m3 = ml_dtypes.float8_e4m3fn
    bf16 = ml_dtypes.bfloat16
    im = np.ascontiguousarray(im, dtype=np.float32)
    s = np.ascontiguousarray(s, dtype=np.float32)
    n = np.asarray(s_l).astype(np.int64)

    caps_of_core = assign_captions(s_l)
    packed = [int(sum(n[i] for i in caps)) for caps in caps_of_core]
    wt_tiles = max(1, -(-max(packed) // 128))  # ceil to 128
    wpad = wt_tiles * 128

    # [p, k, n] with n = j*R + r
    imr = np.ascontiguousarray(
        im.reshape(B * R, KT, 128).transpose(2, 1, 0)).astype(e4m3)
    pieces = []
    n0 = 0
    for G in IM_SPLIT:
        cols = G * R
        pieces.append(np.ascontiguousarray(
            imr[:, :, n0:n0 + cols]).reshape(128, KT * cols))
        n0 += cols
    im_lin = np.ascontiguousarray(np.concatenate(pieces, axis=1))

    in_maps = []
    for c in range(N_CORES):
        sT = np.zeros((D, wpad), dtype=np.float32)
        mask = np.zeros((wpad, 128), dtype=np.float32)  # slots padded to 128
        off = 0
        for slot, cap in enumerate(caps_of_core[c]):
            ni = int(n[cap])
            sT[:, off:off + ni] = s[cap, :ni, :].T
            mask[off:off + ni, slot] = 1.0
            off += ni
        # sT8[p, k*wpad + w] = sT[k*128 + p, w]  (e4m3)
        sT8 = np.ascontiguousarray(
            sT.reshape(KT, 128, wpad).transpose(1, 0, 2).reshape(128, -1)
            .astype(e4m3))
        # maskb[p, t*128 + m] = mask[t*128 + p, m]  (bf16 0/1, exact)
        maskb = np.ascontiguousarray(
            mask.reshape(wt_tiles, 128, 128)
            .transpose(1, 0, 2).reshape(128, -1).astype(bf16))
        in_maps.append({"im_lin": im_lin, "sT8": sT8, "maskb": maskb})
    return in_maps, caps_of_core, wt_tiles


def loss_from_sims(sims_all):
    """sims_all: (B, B) with sims[i cap, j img]; returns scalar loss."""
    scores = sims_all.T.astype(np.float64)  # scores[img j, cap i]
    diag = np.diag(scores).copy()
    cost_s = np.maximum(MARGIN + scores - diag[:, None], 0.0)
    cost_im = np.maximum(MARGIN + scores - diag[None, :], 0.0)
    np.fill_diagonal(cost_s, 0.0)
    np.fill_diagonal(cost_im, 0.0)
    return np.array(cost_s.sum() + cost_im.sum(), dtype=np.float32)


def kernel(im, s, s_l, x, _trace=False, _mm_dtype=None):
    from concourse.bass_utils import run_bass_kernel_spmd

    n = np.asarray(s_l).astype(np.int64)
    in_maps, caps_of_core, wt_tiles = make_core_inputs(im, s, s_l)
    nc = get_nc(wt_tiles)
    res = run_bass_kernel_spmd(nc, in_maps, list(range(N_CORES)), trace=_trace)
    sims_all = np.zeros((B, B), dtype=np.float32)
    for c in range(N_CORES):
        block = res.results[c]["out"]  # (16, 128) raw sums in slot order
        for slot, cap in enumerate(caps_of_core[c]):
            sims_all[cap] = block[slot].astype(np.float32) / float(n[cap])
    loss = loss_from_sims(sims_all)
    if _trace:
        return loss, res
    return loss
